# revision 30
# baseline (speedup 1.0000x reference)
"""AdaLN Trainium2 kernel v6.2 — all-DVE stream, pair ops, chain trims.

HW findings driving v6.1 -> v6.2:
  - GPSIMD concurrency degrades DVE 2-port ts modes (0.47us -> 2.33us
    measured); 1-port tt (2x_1p) is immune. Net: GPS sharing the y-pass
    LOSES time. The x stream is now all-DVE; GPSIMD is idle.
  - per-op overhead ~0.15-0.2us: v/y run as 2-tile [128,2,1024] ops.
  - w staged as 8 single-chunk DMAs; |w| sums: ACT chunks 0-5 (Abs+
    accum), DVE chunks 6-7 (tensor_reduce in its idle window).
  - epilogue reordered drain-free; a_row/shift rows emitted bf16 so the
    PE broadcast matmuls run at bf16 speed; a/b broadcasts stored
    doubled ([P,2,D]) for the pair ops.
x path per [128,1024] tile i (stream iter m covers tiles 2m, 2m+1):
  ACT : sq+accum -> ss[:,i] ; grouped Ln/Exp -> inv
  DVE : u[2m],u[2m+1] (ts 4x) ; v-pair[m-1] (tt) ; y-pair[m-2] (tt)
  SP  : group-of-4 DMA out from xa
"""

import sys
from contextlib import ExitStack

import numpy as np

sys.path.insert(0, "/opt/trn_rl_repo")
sys.path.insert(0, "/opt/pypackages")

import ml_dtypes

import concourse.bass as bass
from concourse import mybir
from concourse.bass_utils import run_bass_kernel_spmd

F32 = mybir.dt.float32
BF16 = mybir.dt.bfloat16
ALU = mybir.AluOpType
ACTF = mybir.ActivationFunctionType

P = 128
D = 1024
CD = 1024
DD = 2 * D
B = 8
S_FULL = 4096

EPS_RMS = 1e-6
EPS_Q = 1e-5
MAGIC = 1.5 * 2.0**23
WCLIP = 1.25

NWQ = 4  # wq ring depth (chunks in flight to PE)
KC = CD // P  # 8 weight chunks of [128, 2048]
NACTW = 4  # |w| chunks on ACT (rest on DVE)
NXD = 4  # x DMAs (8 tiles each)
GRP = 4  # tiles per rsqrt group / out-DMA group
BC_AT = 5  # sq group after which ACT runs the a/b broadcast copies
NU = 12  # ut ring (tiles)
NVP = 2  # vt ring (pairs)


def build(S=S_FULL):
    NT = S // P  # 32 tiles
    NG = NT // GRP  # 8 groups
    NM = NT // 2  # 16 pair iters
    nc = bass.Bass()

    x_d = nc.declare_dram_parameter("x", [S, D], BF16, isOutput=False)
    c_d = nc.declare_dram_parameter("c", [CD], F32, isOutput=False)
    wt_d = nc.declare_dram_parameter("wt", [CD, DD], BF16, isOutput=False)
    b_d = nc.declare_dram_parameter("b", [DD], F32, isOutput=False)
    g_d = nc.declare_dram_parameter("g", [D], F32, isOutput=False)
    out_d = nc.declare_dram_parameter("out", [S, D], BF16, isOutput=True)

    ctx = ExitStack()
    with ctx:
        # ---------------- SBUF ----------------
        ones = ctx.enter_context(nc.sbuf_tensor("ones", [P, P], F32))
        ones_bf = ctx.enter_context(nc.sbuf_tensor("ones_bf", [1, P], BF16))
        eps_t = ctx.enter_context(nc.sbuf_tensor("eps", [P, 1], F32))
        xa = ctx.enter_context(nc.sbuf_tensor("xa", [P, NT, D], BF16))
        wt_sb = ctx.enter_context(nc.sbuf_tensor("wt_sb", [P, KC, DD], BF16))
        wq = [
            ctx.enter_context(nc.sbuf_tensor(f"wq{j}", [P, DD], BF16))
            for j in range(NWQ)
        ]
        tq = [
            ctx.enter_context(nc.sbuf_tensor(f"tq{j}", [P, DD], F32))
            for j in range(3)
        ]
        sqscr = ctx.enter_context(nc.sbuf_tensor("sqscr", [P, D], BF16))
        ut = ctx.enter_context(nc.sbuf_tensor("ut", [P, NU, D], BF16))
        vt = ctx.enter_context(nc.sbuf_tensor("vt", [P, NVP, 2, D], BF16))
        red = ctx.enter_context(nc.sbuf_tensor("red", [P, KC], F32))
        sw = ctx.enter_context(nc.sbuf_tensor("sw", [P, 1], F32))
        swa = ctx.enter_context(nc.sbuf_tensor("swa", [P, 1], F32))
        mp = ctx.enter_context(nc.sbuf_tensor("mp", [P, 1], F32))
        swinv = ctx.enter_context(nc.sbuf_tensor("swinv", [P, 1], F32))
        c_row = ctx.enter_context(nc.sbuf_tensor("c_row", [1, CD], F32))
        ct = ctx.enter_context(nc.sbuf_tensor("ct", [P, KC], F32))
        am = ctx.enter_context(nc.sbuf_tensor("am", [1, 1], F32))
        amc = ctx.enter_context(nc.sbuf_tensor("amc", [1, 1], F32))
        rc = ctx.enter_context(nc.sbuf_tensor("rc", [1, 1], F32))
        r127 = ctx.enter_context(nc.sbuf_tensor("r127", [1, 1], F32))
        r127_b = ctx.enter_context(nc.sbuf_tensor("r127_b", [P, 1], F32))
        cqt = ctx.enter_context(nc.sbuf_tensor("cqt", [P, KC], F32))
        cqi = ctx.enter_context(nc.sbuf_tensor("cqi", [P, KC], BF16))
        osx = ctx.enter_context(nc.sbuf_tensor("osx", [1, 1], F32))
        os_t = ctx.enter_context(nc.sbuf_tensor("os_t", [1, 1], F32))
        b_row = ctx.enter_context(nc.sbuf_tensor("b_row", [1, DD], F32))
        g_row = ctx.enter_context(nc.sbuf_tensor("g_row", [1, D], F32))
        emb = ctx.enter_context(nc.sbuf_tensor("emb", [1, DD], F32))
        a_row = ctx.enter_context(nc.sbuf_tensor("a_row", [1, D], BF16))
        sh_row = ctx.enter_context(nc.sbuf_tensor("sh_row", [1, D], BF16))
        a_bc = ctx.enter_context(nc.sbuf_tensor("a_bc", [P, D], BF16))
        b_bc = ctx.enter_context(nc.sbuf_tensor("b_bc", [P, D], BF16))
        ss = ctx.enter_context(nc.sbuf_tensor("ss", [P, NT], F32))
        stdv = ctx.enter_context(nc.sbuf_tensor("stdv", [P, NT], F32))
        inv = ctx.enter_context(nc.sbuf_tensor("inv", [P, NT], F32))

        par_ps = ctx.enter_context(nc.psum_tensor("par_ps", [P, 2], F32))
        emb_ps = ctx.enter_context(nc.psum_tensor("emb_ps", [1, 4, 512], F32))
        bc_ps = [
            ctx.enter_context(nc.psum_tensor(f"bc_ps{j}", [P, 512], F32))
            for j in range(3)
        ]

        # ---------------- semaphores ----------------
        sem_pre = ctx.enter_context(nc.semaphore("pre"))
        sem_vec = ctx.enter_context(nc.semaphore("vec"))
        sem_w = [ctx.enter_context(nc.semaphore(f"w{j}")) for j in range(KC)]
        sem_x = [ctx.enter_context(nc.semaphore(f"x{j}")) for j in range(NXD)]
        sem_red = ctx.enter_context(nc.semaphore("reds"))
        sem_sw = ctx.enter_context(nc.semaphore("sws"))
        sem_swcp = ctx.enter_context(nc.semaphore("swcp"))
        sem_pe1 = ctx.enter_context(nc.semaphore("pe1"))
        sem_r127 = ctx.enter_context(nc.semaphore("r127s"))
        sem_r127cp = ctx.enter_context(nc.semaphore("r127cp"))
        sem_cq = ctx.enter_context(nc.semaphore("cqs"))
        sem_wq = ctx.enter_context(nc.semaphore("wqs"))
        sem_mmk = ctx.enter_context(nc.semaphore("mmk"))
        sem_mm7a = ctx.enter_context(nc.semaphore("mm7a"))
        sem_emb = ctx.enter_context(nc.semaphore("embs"))
        sem_emb2 = ctx.enter_context(nc.semaphore("embs2"))
        sem_bcmm = ctx.enter_context(nc.semaphore("bcmm"))
        sem_bccp = ctx.enter_context(nc.semaphore("bccp"))
        sem_inv = ctx.enter_context(nc.semaphore("invs"))
        sem_vp = ctx.enter_context(nc.semaphore("vps"))
        sem_uact = ctx.enter_context(nc.semaphore("uacts"))
        sem_yg = [ctx.enter_context(nc.semaphore(f"yg{m}")) for m in range(NG)]
        sem_og = [ctx.enter_context(nc.semaphore(f"og{m}")) for m in range(NG)]

        with nc.Block() as block:

            # ================= SP =================
            @block.sync
            def _(sync):
                sync.dma_start(out=c_row[:], in_=c_d[None, :]).then_inc(sem_vec, 16)
                with nc.allow_non_contiguous_dma(reason="tiny 4KB c chunk load"):
                    sync.dma_start(
                        out=ct[:], in_=c_d[:].rearrange("(k p) -> p k", p=P)
                    ).then_inc(sem_vec, 16)
                for k in range(KC):
                    sync.dma_start(
                        out=wt_sb[:, k, :],
                        in_=wt_d[P * k : P * (k + 1), :],
                    ).then_inc(sem_w[k], 16)
                sync.dma_start(out=b_row[:], in_=b_d[None, :]).then_inc(sem_vec, 16)
                sync.dma_start(out=g_row[:], in_=g_d[None, :]).then_inc(sem_vec, 16)
                tpd = NT // NXD  # tiles per x dma
                for j in range(NXD):
                    sync.dma_start(
                        out=xa[:, tpd * j : tpd * (j + 1), :],
                        in_=x_d[tpd * P * j : tpd * P * (j + 1), :].rearrange(
                            "(t p) d -> p t d", p=P
                        ),
                    ).then_inc(sem_x[j], 16)
                for m in range(NG - 2):
                    sync.wait_ge(sem_yg[m], 2)
                    sync.dma_start(
                        out=out_d[GRP * P * m : GRP * P * (m + 1), :].rearrange(
                            "(t p) d -> p t d", p=P
                        ),
                        in_=xa[:, GRP * m : GRP * (m + 1), :],
                    ).then_inc(sem_og[m], 16)
                for m in range(NG - 2, NG):
                    for h in range(2):
                        sync.wait_ge(sem_yg[m], h + 1)
                        t0_ = GRP * m + 2 * h
                        sync.dma_start(
                            out=out_d[P * t0_ : P * (t0_ + 2), :].rearrange(
                                "(t p) d -> p t d", p=P
                            ),
                            in_=xa[:, t0_ : t0_ + 2, :],
                        ).then_inc(sem_og[m], 16)
                for m in range(NG):
                    sync.wait_ge(sem_og[m], 16 * (2 if m >= NG - 2 else 1))

            # ================= DVE =================
            @block.vector
            def _(vector):
                vector.memset(ones[:], 1.0).then_inc(sem_pre, 1)
                vector.memset(eps_t[:], EPS_RMS).then_inc(sem_pre, 1)
                vector.memset(ones_bf[:], 1.0)

                # --- c quant ---
                vector.wait_ge(sem_vec, 32)
                vector.tensor_reduce(
                    out=am[:], in_=c_row[:], axis=mybir.AxisListType.X,
                    op=ALU.max, apply_absolute_value=True,
                )
                vector.drain()
                vector.tensor_scalar(
                    out=amc[:], in0=am[:], scalar1=EPS_Q, scalar2=None, op0=ALU.max
                )
                vector.drain()
                vector.reciprocal(rc[:], amc[:])
                vector.drain()
                vector.tensor_scalar(
                    out=r127[:], in0=rc[:], scalar1=127.0, scalar2=None,
                    op0=ALU.mult,
                ).then_inc(sem_r127, 1)
                vector.wait_ge(sem_pe1, 1)
                vector.tensor_copy(r127_b[:], par_ps[:, 1:2]).then_inc(sem_r127cp, 1)
                vector.drain()
                vector.tensor_scalar(
                    out=cqt[:], in0=ct[:], scalar1=r127_b[:], scalar2=MAGIC,
                    op0=ALU.mult, op1=ALU.add,
                )
                vector.drain()
                vector.tensor_scalar(
                    out=cqi[:], in0=cqt[:], scalar1=MAGIC, scalar2=None,
                    op0=ALU.subtract,
                ).then_inc(sem_cq, 1)

                # --- |w| for chunks NACTW..KC-1 (ACT does 0..NACTW-1) ---
                for k in range(NACTW, KC):
                    vector.wait_ge(sem_w[k], 16)
                    vector.tensor_reduce(
                        out=red[:, k : k + 1], in_=wt_sb[:, k, :],
                        axis=mybir.AxisListType.X, op=ALU.add,
                        apply_absolute_value=True,
                    )
                # --- weight scale ---
                vector.wait_ge(sem_red, NACTW)
                vector.drain()
                vector.tensor_reduce(
                    out=sw[:], in_=red[:], axis=mybir.AxisListType.X, op=ALU.add
                ).then_inc(sem_sw, 1)
                vector.wait_ge(sem_pe1, 2)
                vector.tensor_copy(swa[:], par_ps[:, 0:1]).then_inc(sem_swcp, 1)
                vector.drain()
                vector.tensor_scalar(
                    out=mp[:], in0=swa[:], scalar1=1.0 / (CD * DD), scalar2=EPS_Q,
                    op0=ALU.mult, op1=ALU.max,
                )
                vector.drain()
                vector.reciprocal(swinv[:], mp[:])
                vector.drain()
                vector.tensor_tensor(osx[:], amc[:], mp[0:1, :], op=ALU.mult)
                vector.drain()
                vector.tensor_scalar(
                    out=os_t[:], in0=osx[:], scalar1=1.0 / 127.0, scalar2=None,
                    op0=ALU.mult,
                )

                # --- w quant: chunk pairs interleaved, drain-free ---
                for kp in range(0, KC, 2):
                    if kp >= NWQ:
                        vector.wait_ge(sem_mmk, kp - NWQ + 2)
                    # A=tq0 B=tq1 C=tq2; WAR reuse of A is in-order-safe
                    vector.tensor_scalar(
                        out=tq[0][:], in0=wt_sb[:, kp, :], scalar1=swinv[:],
                        scalar2=WCLIP, op0=ALU.mult, op1=ALU.min,
                    )
                    vector.tensor_scalar(
                        out=tq[1][:], in0=wt_sb[:, kp + 1, :], scalar1=swinv[:],
                        scalar2=WCLIP, op0=ALU.mult, op1=ALU.min,
                    )
                    vector.tensor_scalar(
                        out=tq[2][:], in0=tq[0][:], scalar1=-WCLIP,
                        scalar2=MAGIC, op0=ALU.max, op1=ALU.add,
                    )
                    vector.tensor_scalar(
                        out=tq[0][:], in0=tq[1][:], scalar1=-WCLIP,
                        scalar2=MAGIC, op0=ALU.max, op1=ALU.add,
                    )
                    vector.tensor_scalar(
                        out=wq[kp % NWQ][:], in0=tq[2][:], scalar1=MAGIC,
                        scalar2=None, op0=ALU.subtract,
                    ).then_inc(sem_wq, 1)
                    vector.tensor_scalar(
                        out=wq[(kp + 1) % NWQ][:], in0=tq[0][:], scalar1=MAGIC,
                        scalar2=None, op0=ALU.subtract,
                    ).then_inc(sem_wq, 1)

                # --- emb epilogue (drain-free: RAW distances >= 2) ---
                vector.wait_ge(sem_mm7a, 1)
                vector.wait_ge(sem_vec, 64)
                vector.scalar_tensor_tensor(
                    out=emb[:, 0:D].rearrange("p (n c) -> p n c", n=2),
                    in0=emb_ps[:, 0:2, :], scalar=os_t[:],
                    in1=b_row[:, 0:D].rearrange("p (n c) -> p n c", n=2),
                    op0=ALU.mult, op1=ALU.add,
                )
                vector.wait_ge(sem_mmk, KC)
                vector.scalar_tensor_tensor(
                    out=emb[:, D:DD].rearrange("p (n c) -> p n c", n=2),
                    in0=emb_ps[:, 2:4, :], scalar=os_t[:],
                    in1=b_row[:, D:DD].rearrange("p (n c) -> p n c", n=2),
                    op0=ALU.mult, op1=ALU.add,
                )
                vector.scalar_tensor_tensor(
                    out=a_row[:], in0=emb[:, 0:D], scalar=1.0, in1=g_row[:],
                    op0=ALU.add, op1=ALU.mult,
                ).then_inc(sem_emb, 1)
                vector.tensor_scalar(
                    out=sh_row[:], in0=emb[:, D:DD], scalar1=1.0, scalar2=None,
                    op0=ALU.mult,
                ).then_inc(sem_emb2, 1)

                # --- u-prefetch (12 tiles): fills the bcast handshake wait ---
                for j in range(0, 12):
                    if j % GRP == 0:
                        vector.wait_ge(sem_inv, j // GRP + 1)
                    vector.tensor_scalar(
                        out=ut[:, j % NU, :], in0=xa[:, j, :],
                        scalar1=inv[:, j : j + 1], scalar2=None, op0=ALU.mult,
                    )

                # --- x stream: pair iters; ACT owns u for tiles >= 24 ---
                for m in range(NM + 2):
                    if m < NM:
                        for j in (2 * m, 2 * m + 1):
                            if j < 12 or j >= 16:
                                continue
                            if j % GRP == 0:
                                vector.wait_ge(sem_inv, j // GRP + 1)
                            vector.tensor_scalar(
                                out=ut[:, j % NU, :], in0=xa[:, j, :],
                                scalar1=inv[:, j : j + 1], scalar2=None,
                                op0=ALU.mult,
                            )
                    if m == 0:
                        vector.wait_ge(sem_bccp, 2)
                    if 1 <= m <= NM:
                        mm_ = m - 1
                        if mm_ >= 8:
                            vector.wait_ge(sem_uact, mm_ - 7)
                        u0 = (2 * mm_) % NU
                        vector.tensor_tensor(
                            out=vt[:, mm_ % NVP, :, :],
                            in0=ut[:, u0 : u0 + 2, :],
                            in1=a_bc[:, None, :].broadcast_to([P, 2, D]),
                            op=ALU.mult,
                        ).then_inc(sem_vp, 1)
                    if m == 1:
                        vector.wait_ge(sem_bccp, 4)
                    if m >= 2:
                        mm_ = m - 2
                        vector.tensor_tensor(
                            out=xa[:, 2 * mm_ : 2 * mm_ + 2, :],
                            in0=vt[:, mm_ % NVP, :, :],
                            in1=b_bc[:, None, :].broadcast_to([P, 2, D]),
                            op=ALU.add,
                        ).then_inc(sem_yg[mm_ // 2], 1)

            # ================= ACT =================
            @block.scalar
            def _(scalar):
                scalar.wait_ge(sem_pre, 2)
                # |w| chunk sums while w DMAs land
                for k in range(NACTW):
                    scalar.wait_ge(sem_w[k], 16)
                    scalar.activation(
                        wq[0][:], wt_sb[:, k, :], ACTF.Abs,
                        accum_out=red[:, k : k + 1],
                    ).then_inc(sem_red, 1)

                def bcast_copies():
                    for jj in range(2):
                        scalar.wait_ge(sem_bcmm, jj + 1)
                        sl = slice(jj * 512, (jj + 1) * 512)
                        scalar.copy(a_bc[:, sl], bc_ps[jj][:, :]).then_inc(
                            sem_bccp, 1
                        )
                    for jj, bank in ((0, 2), (1, 0)):
                        scalar.wait_ge(sem_bcmm, 3 + jj)
                        sl = slice(jj * 512, (jj + 1) * 512)
                        scalar.copy(b_bc[:, sl], bc_ps[bank][:, :]).then_inc(
                            sem_bccp, 1
                        )

                for g in range(NG + 2):
                    if g < NG:
                        for t in range(GRP):
                            i = GRP * g + t
                            if i % (NT // NXD) == 0:
                                scalar.wait_ge(sem_x[i // (NT // NXD)], 16)
                            scalar.activation(
                                sqscr[:], xa[:, i, :], ACTF.Square,
                                accum_out=ss[:, i : i + 1],
                            )
                    if g == BC_AT:
                        bcast_copies()
                    if 1 <= g <= NG:
                        gg = g - 1
                        sl = slice(GRP * gg, GRP * (gg + 1))
                        scalar.activation(
                            stdv[:, sl], ss[:, sl], ACTF.Ln,
                            bias=eps_t[:], scale=1.0 / D,
                        )
                    if g >= 2:
                        gg = g - 2
                        sl = slice(GRP * gg, GRP * (gg + 1))
                        scalar.activation(
                            inv[:, sl], stdv[:, sl], ACTF.Exp, scale=-0.5,
                        ).then_inc(sem_inv, 1)

                # --- u for tiles 24..31 (DVE ring handoff via sem_vp) ---
                for j in range(16, NT):
                    if j % 2 == 0:
                        scalar.wait_ge(sem_vp, j // 2 - 5)
                    ua = scalar.activation(
                        ut[:, j % NU, :], xa[:, j, :], ACTF.Copy,
                        scale=inv[:, j : j + 1],
                    )
                    if j % 2 == 1:
                        ua.then_inc(sem_uact, 1)

            # ================= PE =================

            @block.tensor
            def _(tensor):
                tensor.wait_ge(sem_pre, 1)
                tensor.wait_ge(sem_r127, 1)
                tensor.matmul(
                    par_ps[:, 1:2], lhsT=ones[0:1, :], rhs=r127[:],
                    start=True, stop=True,
                ).then_inc(sem_pe1, 1)
                tensor.wait_ge(sem_r127cp, 1)
                tensor.wait_ge(sem_sw, 1)
                tensor.matmul(
                    par_ps[:, 0:1], lhsT=ones[:], rhs=sw[:], start=True, stop=True
                ).then_inc(sem_pe1, 1)
                tensor.wait_ge(sem_cq, 1)
                for k in range(KC):
                    tensor.wait_ge(sem_wq, k + 1)
                    for n in range(4):
                        mmi = tensor.matmul(
                            emb_ps[:, n, :],
                            lhsT=cqi[:, k : k + 1],
                            rhs=wq[k % NWQ][:, n * 512 : (n + 1) * 512],
                            start=(k == 0),
                            stop=(k == KC - 1),
                        )
                        if n == 3:
                            mmi.then_inc(sem_mmk, 1)
                        elif n == 1 and k == KC - 1:
                            mmi.then_inc(sem_mm7a, 1)
                tensor.wait_ge(sem_emb, 1)
                tensor.matmul(
                    bc_ps[0][:], lhsT=ones_bf[:], rhs=a_row[:, 0:512],
                    start=True, stop=True,
                ).then_inc(sem_bcmm, 1)
                tensor.matmul(
                    bc_ps[1][:], lhsT=ones_bf[:], rhs=a_row[:, 512:1024],
                    start=True, stop=True,
                ).then_inc(sem_bcmm, 1)
                tensor.wait_ge(sem_emb2, 1)
                tensor.matmul(
                    bc_ps[2][:], lhsT=ones_bf[:], rhs=sh_row[:, 0:512],
                    start=True, stop=True,
                ).then_inc(sem_bcmm, 1)
                tensor.wait_ge(sem_bccp, 1)
                tensor.matmul(
                    bc_ps[0][:], lhsT=ones_bf[:], rhs=sh_row[:, 512:1024],
                    start=True, stop=True,
                ).then_inc(sem_bcmm, 1)

    return nc


_CACHE = {}


def _built(S=S_FULL):
    key = ("nc", S)
    if key not in _CACHE:
        _CACHE[key] = build(S)
    return _CACHE[key]


def kernel(x, c, w_proj, b_proj, rms_weight, _trace=False):
    x = np.asarray(x)
    c = np.ascontiguousarray(np.asarray(c, dtype=np.float32))
    w_proj = np.asarray(w_proj, dtype=np.float32)
    b_proj = np.ascontiguousarray(np.asarray(b_proj, dtype=np.float32))
    rms_weight = np.ascontiguousarray(np.asarray(rms_weight, dtype=np.float32))

    nc = _built(x.shape[1])
    wt = np.ascontiguousarray(w_proj.T.astype(ml_dtypes.bfloat16))
    xb = np.ascontiguousarray(x.astype(ml_dtypes.bfloat16))

    in_maps = [
        {"x": xb[i], "c": c[i], "wt": wt, "b": b_proj, "g": rms_weight}
        for i in range(B)
    ]
    res = run_bass_kernel_spmd(nc, in_maps, list(range(B)), trace=_trace)
    kernel.last_results = res
    kernel.last_exec_time_ns = res.exec_time_ns
    return np.stack(
        [res.results[i]["out"].astype(np.float32) for i in range(B)], axis=0
    )


# revision 31
# speedup vs baseline: 1.0323x; 1.0323x over previous
"""AdaLN Trainium2 kernel v6.2 — all-DVE stream, pair ops, chain trims.

HW findings driving v6.1 -> v6.2:
  - GPSIMD concurrency degrades DVE 2-port ts modes (0.47us -> 2.33us
    measured); 1-port tt (2x_1p) is immune. Net: GPS sharing the y-pass
    LOSES time. The x stream is now all-DVE; GPSIMD is idle.
  - per-op overhead ~0.15-0.2us: v/y run as 2-tile [128,2,1024] ops.
  - w staged as 8 single-chunk DMAs; |w| sums: ACT chunks 0-5 (Abs+
    accum), DVE chunks 6-7 (tensor_reduce in its idle window).
  - epilogue reordered drain-free; a_row/shift rows emitted bf16 so the
    PE broadcast matmuls run at bf16 speed; a/b broadcasts stored
    doubled ([P,2,D]) for the pair ops.
x path per [128,1024] tile i (stream iter m covers tiles 2m, 2m+1):
  ACT : sq+accum -> ss[:,i] ; grouped Ln/Exp -> inv
  DVE : u[2m],u[2m+1] (ts 4x) ; v-pair[m-1] (tt) ; y-pair[m-2] (tt)
  SP  : group-of-4 DMA out from xa
"""

import sys
from contextlib import ExitStack

import numpy as np

sys.path.insert(0, "/opt/trn_rl_repo")
sys.path.insert(0, "/opt/pypackages")

import ml_dtypes

import concourse.bass as bass
from concourse import mybir
from concourse.bass_utils import run_bass_kernel_spmd

F32 = mybir.dt.float32
BF16 = mybir.dt.bfloat16
ALU = mybir.AluOpType
ACTF = mybir.ActivationFunctionType

P = 128
D = 1024
CD = 1024
DD = 2 * D
B = 8
S_FULL = 4096

EPS_RMS = 1e-6
EPS_Q = 1e-5
MAGIC = 1.5 * 2.0**23
WCLIP = 1.25

NWQ = 4  # wq ring depth (chunks in flight to PE)
KC = CD // P  # 8 weight chunks of [128, 2048]
NACTW = 4  # |w| chunks on ACT (rest on DVE)
NXD = 4  # x DMAs (8 tiles each)
GRP = 4  # tiles per rsqrt group / out-DMA group
BC_AT = 5  # sq group after which ACT runs the a/b broadcast copies
NU = 12  # ut ring (tiles)
NVP = 2  # vt ring (pairs)


def build(S=S_FULL):
    NT = S // P  # 32 tiles
    NG = NT // GRP  # 8 groups
    NM = NT // 2  # 16 pair iters
    nc = bass.Bass()

    x_d = nc.declare_dram_parameter("x", [S, D], BF16, isOutput=False)
    c_d = nc.declare_dram_parameter("c", [CD], F32, isOutput=False)
    wt_d = nc.declare_dram_parameter("wt", [CD, DD], BF16, isOutput=False)
    b_d = nc.declare_dram_parameter("b", [DD], F32, isOutput=False)
    g_d = nc.declare_dram_parameter("g", [D], F32, isOutput=False)
    out_d = nc.declare_dram_parameter("out", [S, D], BF16, isOutput=True)

    ctx = ExitStack()
    with ctx:
        # ---------------- SBUF ----------------
        ones = ctx.enter_context(nc.sbuf_tensor("ones", [P, P], F32))
        ones_bf = ctx.enter_context(nc.sbuf_tensor("ones_bf", [1, P], BF16))
        eps_t = ctx.enter_context(nc.sbuf_tensor("eps", [P, 1], F32))
        xa = ctx.enter_context(nc.sbuf_tensor("xa", [P, NT, D], BF16))
        wt_sb = ctx.enter_context(nc.sbuf_tensor("wt_sb", [P, KC, DD], BF16))
        wq = [
            ctx.enter_context(nc.sbuf_tensor(f"wq{j}", [P, DD], BF16))
            for j in range(NWQ)
        ]
        tq = [
            ctx.enter_context(nc.sbuf_tensor(f"tq{j}", [P, DD], F32))
            for j in range(3)
        ]
        sqscr = ctx.enter_context(nc.sbuf_tensor("sqscr", [P, D], BF16))
        ut = ctx.enter_context(nc.sbuf_tensor("ut", [P, NU, D], BF16))
        vt = ctx.enter_context(nc.sbuf_tensor("vt", [P, NVP, 2, D], BF16))
        red = ctx.enter_context(nc.sbuf_tensor("red", [P, KC], F32))
        sw = ctx.enter_context(nc.sbuf_tensor("sw", [P, 1], F32))
        swa = ctx.enter_context(nc.sbuf_tensor("swa", [P, 1], F32))
        mp = ctx.enter_context(nc.sbuf_tensor("mp", [P, 1], F32))
        swinv = ctx.enter_context(nc.sbuf_tensor("swinv", [P, 1], F32))
        c_row = ctx.enter_context(nc.sbuf_tensor("c_row", [1, CD], F32))
        ct = ctx.enter_context(nc.sbuf_tensor("ct", [P, KC], F32))
        am = ctx.enter_context(nc.sbuf_tensor("am", [1, 1], F32))
        amc = ctx.enter_context(nc.sbuf_tensor("amc", [1, 1], F32))
        rc = ctx.enter_context(nc.sbuf_tensor("rc", [1, 1], F32))
        r127 = ctx.enter_context(nc.sbuf_tensor("r127", [1, 1], F32))
        r127_b = ctx.enter_context(nc.sbuf_tensor("r127_b", [P, 1], F32))
        cqt = ctx.enter_context(nc.sbuf_tensor("cqt", [P, KC], F32))
        cqi = ctx.enter_context(nc.sbuf_tensor("cqi", [P, KC], BF16))
        osx = ctx.enter_context(nc.sbuf_tensor("osx", [1, 1], F32))
        os_t = ctx.enter_context(nc.sbuf_tensor("os_t", [1, 1], F32))
        b_row = ctx.enter_context(nc.sbuf_tensor("b_row", [1, DD], F32))
        g_row = ctx.enter_context(nc.sbuf_tensor("g_row", [1, D], F32))
        emb = ctx.enter_context(nc.sbuf_tensor("emb", [1, DD], F32))
        a_row = ctx.enter_context(nc.sbuf_tensor("a_row", [1, D], BF16))
        sh_row = ctx.enter_context(nc.sbuf_tensor("sh_row", [1, D], BF16))
        a_bc = ctx.enter_context(nc.sbuf_tensor("a_bc", [P, D], BF16))
        b_bc = ctx.enter_context(nc.sbuf_tensor("b_bc", [P, D], BF16))
        ss = ctx.enter_context(nc.sbuf_tensor("ss", [P, NT], F32))
        stdv = ctx.enter_context(nc.sbuf_tensor("stdv", [P, NT], F32))
        inv = ctx.enter_context(nc.sbuf_tensor("inv", [P, NT], F32))

        par_ps = ctx.enter_context(nc.psum_tensor("par_ps", [P, 2], F32))
        emb_ps = ctx.enter_context(nc.psum_tensor("emb_ps", [1, 4, 512], F32))
        bc_ps = [
            ctx.enter_context(nc.psum_tensor(f"bc_ps{j}", [P, 512], F32))
            for j in range(3)
        ]

        # ---------------- semaphores ----------------
        sem_pre = ctx.enter_context(nc.semaphore("pre"))
        sem_vec = ctx.enter_context(nc.semaphore("vec"))
        sem_w = [ctx.enter_context(nc.semaphore(f"w{j}")) for j in range(KC)]
        sem_x = [ctx.enter_context(nc.semaphore(f"x{j}")) for j in range(NXD)]
        sem_red = ctx.enter_context(nc.semaphore("reds"))
        sem_sw = ctx.enter_context(nc.semaphore("sws"))
        sem_swcp = ctx.enter_context(nc.semaphore("swcp"))
        sem_pe1 = ctx.enter_context(nc.semaphore("pe1"))
        sem_r127 = ctx.enter_context(nc.semaphore("r127s"))
        sem_r127cp = ctx.enter_context(nc.semaphore("r127cp"))
        sem_cq = ctx.enter_context(nc.semaphore("cqs"))
        sem_wq = ctx.enter_context(nc.semaphore("wqs"))
        sem_mmk = ctx.enter_context(nc.semaphore("mmk"))
        sem_mm7a = ctx.enter_context(nc.semaphore("mm7a"))
        sem_emb = ctx.enter_context(nc.semaphore("embs"))
        sem_emb2 = ctx.enter_context(nc.semaphore("embs2"))
        sem_bcmm = ctx.enter_context(nc.semaphore("bcmm"))
        sem_bccp = ctx.enter_context(nc.semaphore("bccp"))
        sem_inv = ctx.enter_context(nc.semaphore("invs"))
        sem_vp = ctx.enter_context(nc.semaphore("vps"))
        sem_uact = ctx.enter_context(nc.semaphore("uacts"))
        sem_yg = [ctx.enter_context(nc.semaphore(f"yg{m}")) for m in range(NG)]
        sem_og = [ctx.enter_context(nc.semaphore(f"og{m}")) for m in range(NG)]

        with nc.Block() as block:

            # ================= SP =================
            @block.sync
            def _(sync):
                sync.dma_start(out=c_row[:], in_=c_d[None, :]).then_inc(sem_vec, 16)
                with nc.allow_non_contiguous_dma(reason="tiny 4KB c chunk load"):
                    sync.dma_start(
                        out=ct[:], in_=c_d[:].rearrange("(k p) -> p k", p=P)
                    ).then_inc(sem_vec, 16)
                for k in range(KC):
                    sync.dma_start(
                        out=wt_sb[:, k, :],
                        in_=wt_d[P * k : P * (k + 1), :],
                    ).then_inc(sem_w[k], 16)
                sync.dma_start(out=b_row[:], in_=b_d[None, :]).then_inc(sem_vec, 16)
                sync.dma_start(out=g_row[:], in_=g_d[None, :]).then_inc(sem_vec, 16)
                tpd = NT // NXD  # tiles per x dma
                for j in range(NXD):
                    sync.dma_start(
                        out=xa[:, tpd * j : tpd * (j + 1), :],
                        in_=x_d[tpd * P * j : tpd * P * (j + 1), :].rearrange(
                            "(t p) d -> p t d", p=P
                        ),
                    ).then_inc(sem_x[j], 16)
                for m in range(NG - 2):
                    sync.wait_ge(sem_yg[m], 2)
                    sync.dma_start(
                        out=out_d[GRP * P * m : GRP * P * (m + 1), :].rearrange(
                            "(t p) d -> p t d", p=P
                        ),
                        in_=xa[:, GRP * m : GRP * (m + 1), :],
                    ).then_inc(sem_og[m], 16)
                for m in range(NG - 2, NG):
                    for h in range(2):
                        sync.wait_ge(sem_yg[m], h + 1)
                        t0_ = GRP * m + 2 * h
                        sync.dma_start(
                            out=out_d[P * t0_ : P * (t0_ + 2), :].rearrange(
                                "(t p) d -> p t d", p=P
                            ),
                            in_=xa[:, t0_ : t0_ + 2, :],
                        ).then_inc(sem_og[m], 16)
                for m in range(NG):
                    sync.wait_ge(sem_og[m], 16 * (2 if m >= NG - 2 else 1))

            # ================= DVE =================
            @block.vector
            def _(vector):
                vector.memset(ones[:], 1.0).then_inc(sem_pre, 1)
                vector.memset(eps_t[:], EPS_RMS).then_inc(sem_pre, 1)
                vector.memset(ones_bf[:], 1.0)

                # --- c quant ---
                vector.wait_ge(sem_vec, 32)
                vector.tensor_reduce(
                    out=am[:], in_=c_row[:], axis=mybir.AxisListType.X,
                    op=ALU.max, apply_absolute_value=True,
                )
                vector.drain()
                vector.tensor_scalar(
                    out=amc[:], in0=am[:], scalar1=EPS_Q, scalar2=None, op0=ALU.max
                )
                vector.drain()
                vector.reciprocal(rc[:], amc[:])
                vector.drain()
                vector.tensor_scalar(
                    out=r127[:], in0=rc[:], scalar1=127.0, scalar2=None,
                    op0=ALU.mult,
                ).then_inc(sem_r127, 1)
                vector.wait_ge(sem_pe1, 1)
                vector.tensor_copy(r127_b[:], par_ps[:, 1:2]).then_inc(sem_r127cp, 1)
                vector.drain()
                vector.tensor_scalar(
                    out=cqt[:], in0=ct[:], scalar1=r127_b[:], scalar2=MAGIC,
                    op0=ALU.mult, op1=ALU.add,
                )
                vector.drain()
                vector.tensor_scalar(
                    out=cqi[:], in0=cqt[:], scalar1=MAGIC, scalar2=None,
                    op0=ALU.subtract,
                ).then_inc(sem_cq, 1)

                # --- |w| for chunks NACTW..KC-1 (ACT does 0..NACTW-1) ---
                for k in range(NACTW, KC):
                    vector.wait_ge(sem_w[k], 16)
                    vector.tensor_reduce(
                        out=red[:, k : k + 1], in_=wt_sb[:, k, :],
                        axis=mybir.AxisListType.X, op=ALU.add,
                        apply_absolute_value=True,
                    )
                # --- weight scale ---
                vector.wait_ge(sem_red, NACTW)
                vector.drain()
                vector.tensor_reduce(
                    out=sw[:], in_=red[:], axis=mybir.AxisListType.X, op=ALU.add
                ).then_inc(sem_sw, 1)
                vector.wait_ge(sem_pe1, 2)
                vector.tensor_copy(swa[:], par_ps[:, 0:1]).then_inc(sem_swcp, 1)
                vector.drain()
                vector.tensor_scalar(
                    out=mp[:], in0=swa[:], scalar1=1.0 / (CD * DD), scalar2=EPS_Q,
                    op0=ALU.mult, op1=ALU.max,
                )
                vector.drain()
                vector.reciprocal(swinv[:], mp[:])
                vector.drain()
                vector.tensor_tensor(osx[:], amc[:], mp[0:1, :], op=ALU.mult)
                vector.drain()
                vector.tensor_scalar(
                    out=os_t[:], in0=osx[:], scalar1=1.0 / 127.0, scalar2=None,
                    op0=ALU.mult,
                )

                # --- w quant: chunk pairs interleaved, drain-free ---
                for kp in range(0, KC, 2):
                    if kp >= NWQ:
                        vector.wait_ge(sem_mmk, kp - NWQ + 2)
                    # A=tq0 B=tq1 C=tq2; WAR reuse of A is in-order-safe
                    vector.tensor_scalar(
                        out=tq[0][:], in0=wt_sb[:, kp, :], scalar1=swinv[:],
                        scalar2=WCLIP, op0=ALU.mult, op1=ALU.min,
                    )
                    vector.tensor_scalar(
                        out=tq[1][:], in0=wt_sb[:, kp + 1, :], scalar1=swinv[:],
                        scalar2=WCLIP, op0=ALU.mult, op1=ALU.min,
                    )
                    vector.tensor_scalar(
                        out=tq[2][:], in0=tq[0][:], scalar1=-WCLIP,
                        scalar2=MAGIC, op0=ALU.max, op1=ALU.add,
                    )
                    vector.tensor_scalar(
                        out=tq[0][:], in0=tq[1][:], scalar1=-WCLIP,
                        scalar2=MAGIC, op0=ALU.max, op1=ALU.add,
                    )
                    vector.tensor_scalar(
                        out=wq[kp % NWQ][:], in0=tq[2][:], scalar1=MAGIC,
                        scalar2=None, op0=ALU.subtract,
                    ).then_inc(sem_wq, 1)
                    vector.tensor_scalar(
                        out=wq[(kp + 1) % NWQ][:], in0=tq[0][:], scalar1=MAGIC,
                        scalar2=None, op0=ALU.subtract,
                    ).then_inc(sem_wq, 1)

                # --- u-prefetch while PE finishes the emb matmuls ---
                NPRE = 8
                for j in range(NPRE):
                    if j % GRP == 0:
                        vector.wait_ge(sem_inv, j // GRP + 1)
                    vector.tensor_scalar(
                        out=ut[:, j % NU, :], in0=xa[:, j, :],
                        scalar1=inv[:, j : j + 1], scalar2=None, op0=ALU.mult,
                    )

                # --- emb epilogue (drain-free: RAW distances >= 2) ---
                vector.wait_ge(sem_mm7a, 1)
                vector.wait_ge(sem_vec, 64)
                vector.scalar_tensor_tensor(
                    out=emb[:, 0:D].rearrange("p (n c) -> p n c", n=2),
                    in0=emb_ps[:, 0:2, :], scalar=os_t[:],
                    in1=b_row[:, 0:D].rearrange("p (n c) -> p n c", n=2),
                    op0=ALU.mult, op1=ALU.add,
                )
                vector.wait_ge(sem_mmk, KC)
                vector.scalar_tensor_tensor(
                    out=emb[:, D:DD].rearrange("p (n c) -> p n c", n=2),
                    in0=emb_ps[:, 2:4, :], scalar=os_t[:],
                    in1=b_row[:, D:DD].rearrange("p (n c) -> p n c", n=2),
                    op0=ALU.mult, op1=ALU.add,
                )
                vector.scalar_tensor_tensor(
                    out=a_row[:], in0=emb[:, 0:D], scalar=1.0, in1=g_row[:],
                    op0=ALU.add, op1=ALU.mult,
                ).then_inc(sem_emb, 1)
                vector.tensor_scalar(
                    out=sh_row[:], in0=emb[:, D:DD], scalar1=1.0, scalar2=None,
                    op0=ALU.mult,
                ).then_inc(sem_emb2, 1)

                # --- u-prefetch B: fills the a_bc broadcast wait ---
                for j in range(NPRE, 12):
                    if j % GRP == 0:
                        vector.wait_ge(sem_inv, j // GRP + 1)
                    vector.tensor_scalar(
                        out=ut[:, j % NU, :], in0=xa[:, j, :],
                        scalar1=inv[:, j : j + 1], scalar2=None, op0=ALU.mult,
                    )

                # --- x stream: pair iters; ACT owns u for tiles >= 24 ---
                for m in range(NM + 2):
                    if m < NM:
                        for j in (2 * m, 2 * m + 1):
                            if j < 12 or j >= 16:
                                continue
                            if j % GRP == 0:
                                vector.wait_ge(sem_inv, j // GRP + 1)
                            vector.tensor_scalar(
                                out=ut[:, j % NU, :], in0=xa[:, j, :],
                                scalar1=inv[:, j : j + 1], scalar2=None,
                                op0=ALU.mult,
                            )
                    if m == 0:
                        vector.wait_ge(sem_bccp, 2)
                    if 1 <= m <= NM:
                        mm_ = m - 1
                        if mm_ >= 8:
                            vector.wait_ge(sem_uact, mm_ - 7)
                        u0 = (2 * mm_) % NU
                        vector.tensor_tensor(
                            out=vt[:, mm_ % NVP, :, :],
                            in0=ut[:, u0 : u0 + 2, :],
                            in1=a_bc[:, None, :].broadcast_to([P, 2, D]),
                            op=ALU.mult,
                        ).then_inc(sem_vp, 1)
                    if m == 1:
                        vector.wait_ge(sem_bccp, 4)
                    if m >= 2:
                        mm_ = m - 2
                        vector.tensor_tensor(
                            out=xa[:, 2 * mm_ : 2 * mm_ + 2, :],
                            in0=vt[:, mm_ % NVP, :, :],
                            in1=b_bc[:, None, :].broadcast_to([P, 2, D]),
                            op=ALU.add,
                        ).then_inc(sem_yg[mm_ // 2], 1)

            # ================= ACT =================
            @block.scalar
            def _(scalar):
                scalar.wait_ge(sem_pre, 2)
                # |w| chunk sums while w DMAs land
                for k in range(NACTW):
                    scalar.wait_ge(sem_w[k], 16)
                    scalar.activation(
                        wq[0][:], wt_sb[:, k, :], ACTF.Abs,
                        accum_out=red[:, k : k + 1],
                    ).then_inc(sem_red, 1)

                def bcast_copies():
                    for jj in range(2):
                        scalar.wait_ge(sem_bcmm, jj + 1)
                        sl = slice(jj * 512, (jj + 1) * 512)
                        scalar.copy(a_bc[:, sl], bc_ps[jj][:, :]).then_inc(
                            sem_bccp, 1
                        )
                    for jj, bank in ((0, 2), (1, 0)):
                        scalar.wait_ge(sem_bcmm, 3 + jj)
                        sl = slice(jj * 512, (jj + 1) * 512)
                        scalar.copy(b_bc[:, sl], bc_ps[bank][:, :]).then_inc(
                            sem_bccp, 1
                        )

                for g in range(NG + 2):
                    if g < NG:
                        for t in range(GRP):
                            i = GRP * g + t
                            if i % (NT // NXD) == 0:
                                scalar.wait_ge(sem_x[i // (NT // NXD)], 16)
                            scalar.activation(
                                sqscr[:], xa[:, i, :], ACTF.Square,
                                accum_out=ss[:, i : i + 1],
                            )
                    if g == BC_AT:
                        bcast_copies()
                    if 1 <= g <= NG:
                        gg = g - 1
                        sl = slice(GRP * gg, GRP * (gg + 1))
                        scalar.activation(
                            stdv[:, sl], ss[:, sl], ACTF.Ln,
                            bias=eps_t[:], scale=1.0 / D,
                        )
                    if g >= 2:
                        gg = g - 2
                        sl = slice(GRP * gg, GRP * (gg + 1))
                        scalar.activation(
                            inv[:, sl], stdv[:, sl], ACTF.Exp, scale=-0.5,
                        ).then_inc(sem_inv, 1)

                # --- u for tiles 24..31 (DVE ring handoff via sem_vp) ---
                for j in range(16, NT):
                    if j % 2 == 0:
                        scalar.wait_ge(sem_vp, j // 2 - 5)
                    ua = scalar.activation(
                        ut[:, j % NU, :], xa[:, j, :], ACTF.Copy,
                        scale=inv[:, j : j + 1],
                    )
                    if j % 2 == 1:
                        ua.then_inc(sem_uact, 1)

            # ================= PE =================

            @block.tensor
            def _(tensor):
                tensor.wait_ge(sem_pre, 1)
                tensor.wait_ge(sem_r127, 1)
                tensor.matmul(
                    par_ps[:, 1:2], lhsT=ones[0:1, :], rhs=r127[:],
                    start=True, stop=True,
                ).then_inc(sem_pe1, 1)
                tensor.wait_ge(sem_r127cp, 1)
                tensor.wait_ge(sem_sw, 1)
                tensor.matmul(
                    par_ps[:, 0:1], lhsT=ones[:], rhs=sw[:], start=True, stop=True
                ).then_inc(sem_pe1, 1)
                tensor.wait_ge(sem_cq, 1)
                for k in range(KC):
                    tensor.wait_ge(sem_wq, k + 1)
                    for n in range(4):
                        mmi = tensor.matmul(
                            emb_ps[:, n, :],
                            lhsT=cqi[:, k : k + 1],
                            rhs=wq[k % NWQ][:, n * 512 : (n + 1) * 512],
                            start=(k == 0),
                            stop=(k == KC - 1),
                        )
                        if n == 3:
                            mmi.then_inc(sem_mmk, 1)
                        elif n == 1 and k == KC - 1:
                            mmi.then_inc(sem_mm7a, 1)
                tensor.wait_ge(sem_emb, 1)
                tensor.matmul(
                    bc_ps[0][:], lhsT=ones_bf[:], rhs=a_row[:, 0:512],
                    start=True, stop=True,
                ).then_inc(sem_bcmm, 1)
                tensor.matmul(
                    bc_ps[1][:], lhsT=ones_bf[:], rhs=a_row[:, 512:1024],
                    start=True, stop=True,
                ).then_inc(sem_bcmm, 1)
                tensor.wait_ge(sem_emb2, 1)
                tensor.matmul(
                    bc_ps[2][:], lhsT=ones_bf[:], rhs=sh_row[:, 0:512],
                    start=True, stop=True,
                ).then_inc(sem_bcmm, 1)
                tensor.wait_ge(sem_bccp, 1)
                tensor.matmul(
                    bc_ps[0][:], lhsT=ones_bf[:], rhs=sh_row[:, 512:1024],
                    start=True, stop=True,
                ).then_inc(sem_bcmm, 1)

    return nc


_CACHE = {}


def _built(S=S_FULL):
    key = ("nc", S)
    if key not in _CACHE:
        _CACHE[key] = build(S)
    return _CACHE[key]


def kernel(x, c, w_proj, b_proj, rms_weight, _trace=False):
    x = np.asarray(x)
    c = np.ascontiguousarray(np.asarray(c, dtype=np.float32))
    w_proj = np.asarray(w_proj, dtype=np.float32)
    b_proj = np.ascontiguousarray(np.asarray(b_proj, dtype=np.float32))
    rms_weight = np.ascontiguousarray(np.asarray(rms_weight, dtype=np.float32))

    nc = _built(x.shape[1])
    wt = np.ascontiguousarray(w_proj.T.astype(ml_dtypes.bfloat16))
    xb = np.ascontiguousarray(x.astype(ml_dtypes.bfloat16))

    in_maps = [
        {"x": xb[i], "c": c[i], "wt": wt, "b": b_proj, "g": rms_weight}
        for i in range(B)
    ]
    res = run_bass_kernel_spmd(nc, in_maps, list(range(B)), trace=_trace)
    kernel.last_results = res
    kernel.last_exec_time_ns = res.exec_time_ns
    return np.stack(
        [res.results[i]["out"].astype(np.float32) for i in range(B)], axis=0
    )
